# revision 56
# baseline (speedup 1.0000x reference)
"""GCN layer (fc + gather/scatter-sum) on 8 trn2 NeuronCores.

Math identity used: out = segment_sum(take(feature @ W + b, src), dst)
                        = segment_sum(take(feature, src), dst) @ W + deg * b
so the FC is folded AFTER edge aggregation. Each core owns a contiguous
range of dst nodes; its edges are host-sorted by (dst-block, src-window).
Feature rows are gathered as bf16 (512B rows) from a replicated DRAM table
with the custom SWDGE dma_gather (1 descriptor per edge). Desc-gen at
~8-10ns/desc per Q7 core pair (x4 pairs, one per SWDGE queue) is the
kernel's critical path, so: src windows are sized {27500,27500,22500,
22500} to land per-(block,window) edge counts just under {5,5,4,4} tile
multiples (fewer padded slots than equal windows); the fat windows rotate
across queues per superbatch; per-call num_idxs registers are allocated
per DISTINCT size and loaded once (a per-call reg_mov creates a WAR stall
that serializes the pairs); superbatch 0's idx slice uploads before the
bulk index table. Per 128-dst-node block, a one-hot A[e,d] =
(dst_rel[e] == d) is matmul'ed with the gathered rows into a PSUM
accumulator; A is built per (superbatch, window) cell in one broadcast
DVE is_equal — kept cell-sized because long 2-input DVE ops hold both
SBUF ports of the POOL slot and stall Q7 desc-gen. The block FC runs
on-chip: PE transpose, PSUM->SBUF copies on the otherwise-idle ACT
engine, two 128-contraction matmuls with W, and bias = deg (x) b folded
in as a rank-1 matmul (lhsT = deg row slice, rhs = b row) accumulated
into the same PSUM tile.
"""

import os
import numpy as np
import ml_dtypes

import concourse.bass as bass
import concourse.bacc as bacc
import concourse.mybir as mybir
from concourse import tile
from concourse import library_config

P = 128
NCORES = 8


def _patch_tile_exit():
    """The walrus build in this container rejects two constructs Tile emits
    at TileContext exit: a Drain carrying more than one sync wait ("Too many
    sync wait commands") and the sem_clear InstISA ("ISA wrong length").
    Replace the exit sequence with equivalent one-wait-per-Drain chains and
    skip the semaphore clears (fine for single-execution NEFFs)."""
    import bass_rust
    from concourse.vector_clock import ScopedClock

    def _drain_and_barrier(self, tick_clock, wait_clock):
        drain_inst = self.nc.sync.drain()
        wait_clock.add_sem_waits(
            drain_inst.ins, ScopedClock({None: tick_clock.global_clock})
        )
        si = drain_inst.ins.sync_info
        if si is not None and len(si.on_wait) > 1:
            waits = list(si.on_wait)
            drain_inst.ins.sync_info = bass_rust.SyncInfo(
                on_wait=waits[:1], on_update=list(si.on_update))
            for w in waits[1:]:
                extra = self.nc.sync.drain()
                extra.ins.sync_info = bass_rust.SyncInfo(
                    on_wait=[w], on_update=[])
        self.nc.all_engine_barrier()
        popped = self.nc._tile_sem_poison_stack.pop()
        assert popped is self._sem_poison
        self.nc.all_engine_barrier()

    tile.TileContext._drain_and_barrier = _drain_and_barrier


_patch_tile_exit()


class Cfg:
    def __init__(self, n_nodes, d_in, d_out, ncores, wbounds, sb_blocks):
        self.N = n_nodes
        self.D = d_in
        self.DO = d_out
        self.ncores = ncores
        # src window bounds; sizes tuned so per-(block,window) edge counts
        # land just under tile multiples (5,5,4,4 tiles) -> fewer padded
        # gather slots than equal windows (which ceil to 5 tiles each)
        self.wbounds = wbounds
        self.ngroups = len(wbounds) - 1
        assert max(b - a for a, b in zip(wbounds, wbounds[1:])) <= 32768
        self.npc = n_nodes // ncores       # nodes per core
        self.nblk = (self.npc + P - 1) // P
        self.sb = sb_blocks                # blocks per superbatch (psum + gather call)


FULL_CFG = Cfg(n_nodes=100000, d_in=256, d_out=64, ncores=8,
               wbounds=(0, 27500, 55000, 77500, 100000), sb_blocks=4)


def _prep_host(feature, W, b, src, dst, cfg):
    """Shard + sort edges, build per-core slot arrays and shared tile budgets."""
    N, npc, nblk, ng = cfg.N, cfg.npc, cfg.nblk, cfg.ngroups
    src = np.asarray(src, dtype=np.int64)
    dst = np.asarray(dst, dtype=np.int64)

    per_core = []
    counts = np.zeros((cfg.ncores, nblk, ng), dtype=np.int64)
    for m in range(cfg.ncores):
        lo, hi = m * npc, (m + 1) * npc
        mask = (dst >= lo) & (dst < hi)
        es = src[mask]
        ed = dst[mask] - lo
        blk = ed >> 7
        grp = np.searchsorted(np.asarray(cfg.wbounds), es, side="right") - 1
        order = np.lexsort((es, grp, blk))
        es, ed, blk, grp = es[order], ed[order], blk[order], grp[order]
        np.add.at(counts[m], (blk, grp), 1)
        per_core.append((es, ed, blk, grp))

    # static tile budgets per (block, group): max over cores, >=1 tile per block
    T = np.ceil(counts / P).astype(np.int64).max(axis=0)  # [nblk, ng]
    for k in range(nblk):
        if T[k].sum() == 0:
            T[k][0] = 1

    # slot layout in call order: superbatch -> group -> block -> tiles.
    # The last few superbatches shrink to 2 blocks so the end-of-kernel
    # pipeline drain (compute lags desc-gen by ~2 superbatches) is short.
    sb_ranges = []
    k0 = 0
    while k0 < nblk:
        rem = nblk - k0
        step = cfg.sb if rem > 6 else 2
        sb_ranges.append(range(k0, min(k0 + step, nblk)))
        k0 += step
    nsb = len(sb_ranges)
    call_offsets = {}   # (sb, g) -> (slot_start, slot_count)
    tile_col = {}       # (k, g) -> first slot column (slot_start // P)
    pos = 0
    for s, ks in enumerate(sb_ranges):
        for g in range(ng):
            size = int(sum(T[k][g] for k in ks)) * P
            call_offsets[(s, g)] = (pos, size)
            for k in ks:
                tile_col[(k, g)] = pos // P
                pos += int(T[k][g]) * P
    tot = pos
    assert tot % P == 0

    in_maps = []
    ftab = np.ascontiguousarray(feature.astype(ml_dtypes.bfloat16))
    wmat = np.ascontiguousarray(W.astype(np.float32))
    brow = np.ascontiguousarray(b.astype(ml_dtypes.bfloat16)[None, :])
    iota = np.ascontiguousarray(
        np.tile(np.arange(P, dtype=np.float32)[None, :], (P, 1)))
    ident = np.eye(P, dtype=np.float32)

    for m in range(cfg.ncores):
        es, ed, blk, grp = per_core[m]
        idx_arr = np.zeros(tot, dtype=np.int16)
        dst_arr = np.full(tot, -1.0, dtype=np.float32)
        # edges of (k, g) start at slot tile_col[(k,g)]*P
        bounds = np.searchsorted(blk * ng + grp, np.arange(nblk * ng + 1))
        for k in range(nblk):
            for g in range(ng):
                a, bnd = bounds[k * ng + g], bounds[k * ng + g + 1]
                n = bnd - a
                if n == 0:
                    continue
                s0 = tile_col[(k, g)] * P
                idx_arr[s0:s0 + n] = (es[a:bnd] - cfg.wbounds[g]).astype(np.int16)
                dst_arr[s0:s0 + n] = (ed[a:bnd] - k * P).astype(np.float32)
        # ucode rx/tx cores read the idx wrap from different 16-partition
        # groups (queue-dependent): replicate it across all 128 partitions
        idx16 = np.ascontiguousarray(
            np.tile(idx_arr.reshape(tot // 16, 16).T, (P // 16, 1)))
        dstrel = np.ascontiguousarray(dst_arr.reshape(tot // P, P).T)
        deg = np.zeros(nblk * P, dtype=np.float32)
        np.add.at(deg, ed, 1.0)
        # single partition-0 row; [1,128] lhsT slices per block (bf16 exact
        # for small integer degrees)
        degrow = np.ascontiguousarray(deg[None, :].astype(ml_dtypes.bfloat16))
        in_maps.append({
            "ftab": ftab, "idx16": idx16, "dstrel": dstrel,
            "wmat": wmat, "brow": brow, "iota": iota, "ident": ident,
            "degrow": degrow,
        })

    meta = dict(T=T, call_offsets=call_offsets, tile_col=tile_col, tot=tot,
                nsb=nsb, sb_ranges=sb_ranges)
    return in_maps, meta


def _build_program(cfg, meta):
    N, D, DO, nblk, ng = cfg.N, cfg.D, cfg.DO, cfg.nblk, cfg.ngroups
    T, call_offsets, tile_col = meta["T"], meta["call_offsets"], meta["tile_col"]
    tot, nsb, sb_ranges = meta["tot"], meta["nsb"], meta["sb_ranges"]
    bf16, f32, i16 = mybir.dt.bfloat16, mybir.dt.float32, mybir.dt.int16

    nc = bacc.Bacc(None, target_bir_lowering=False, num_swdge_queues=4)
    ftab = nc.dram_tensor("ftab", [N, D], bf16, kind="ExternalInput")
    idx16 = nc.dram_tensor("idx16", [P, tot // 16], i16, kind="ExternalInput")
    dstrel = nc.dram_tensor("dstrel", [P, tot // P], f32, kind="ExternalInput")
    wmat = nc.dram_tensor("wmat", [D, DO], f32, kind="ExternalInput")
    brow = nc.dram_tensor("brow", [1, DO], bf16, kind="ExternalInput")
    iota = nc.dram_tensor("iota", [P, P], f32, kind="ExternalInput")
    ident = nc.dram_tensor("ident", [P, P], f32, kind="ExternalInput")
    degrow = nc.dram_tensor("degrow", [1, nblk * P], bf16, kind="ExternalInput")
    out = nc.dram_tensor("out", [cfg.npc, DO], f32, kind="ExternalOutput")

    kchunks = D // P  # 128-contraction chunks of the FC

    with tile.TileContext(nc) as tc:
        with (
            tc.tile_pool(name="const", bufs=1) as cpool,
            tc.tile_pool(name="gathf", bufs=3) as gpoolf,
            tc.tile_pool(name="degp", bufs=2) as degp,
            tc.tile_pool(name="amat", bufs=2) as apool,
            tc.tile_pool(name="work", bufs=4) as wpool,
            tc.tile_pool(name="psag", bufs=cfg.sb - 1, space="PSUM") as psag,
            tc.tile_pool(name="pstr", bufs=2, space="PSUM") as pstr,
            tc.tile_pool(name="psout", bufs=2, space="PSUM") as psout,
            tc.tile_pool(name="psio", bufs=1, space="PSUM") as psio,
        ):
            # one Pool register per DISTINCT call size, loaded once up front.
            # A per-call reg_mov would create a WAR hazard on the register
            # (next mov waits for the previous gather reading it), and the
            # in-order Pool FIFO then stalls dispatch of the other queues'
            # gathers — killing the 4-way Q7-pair desc-gen concurrency.
            distinct_sizes = sorted({sz for (_o, sz) in call_offsets.values()
                                     if sz > 0})
            size_regs = {}
            for sz in distinct_sizes:
                r = nc.alloc_register(mybir.EngineType.Pool, f"gsz{sz}")
                nc.gpsimd.reg_mov(r, sz)
                size_regs[sz] = r
            warm_reg = nc.alloc_register(mybir.EngineType.Pool, "gszwarm")
            nc.gpsimd.reg_mov(warm_reg, 16)
            # iota lives in PSUM: the A-build is_equal then needs only one
            # SBUF read port (dstt) + one write (A), so it no longer locks
            # the Q7 desc-gen cores out of the shared POOL SBUF port.
            iotat = cpool.tile([P, P], f32)
            nc.sync.dma_start(out=iotat[:], in_=iota[:])
            iotap = psio.tile([P, P], f32)
            nc.vector.tensor_copy(out=iotap[:], in_=iotat[:])
            identt = cpool.tile([P, P], f32)
            nc.sync.dma_start(out=identt[:], in_=ident[:])

            bt = cpool.tile([1, DO], bf16)
            nc.sync.dma_start(out=bt[:], in_=brow[:])
            wts = []
            for c in range(kchunks):
                wt = cpool.tile([P, DO], f32, tag=f"w{c}")
                nc.sync.dma_start(out=wt[:], in_=wmat[c * P:(c + 1) * P, :])
                wts.append(wt)
            # idx table split: superbatch 0's slice is a separate small tile
            # loaded first so sb0's gathers don't stall behind the full
            # 3.6MB index upload; dstt likewise precedes the bulk idx DMA.
            sb0_end = sum(call_offsets[(0, g)][1] for g in range(ng))
            idxt0 = cpool.tile([P, sb0_end // 16], i16)
            nc.sync.dma_start(out=idxt0[:], in_=idx16[:, 0:sb0_end // 16])
            dstt = cpool.tile([P, tot // P], f32)
            nc.sync.dma_start(out=dstt[:], in_=dstrel[:])
            idxt = cpool.tile([P, tot // 16], i16)
            nc.sync.dma_start(out=idxt[:, sb0_end // 16:],
                              in_=idx16[:, sb0_end // 16:])

            def idx_slice(s, off, size):
                if s == 0:
                    return idxt0[:, off // 16:(off + size) // 16]
                return idxt[:, off // 16:(off + size) // 16]

            # tiny dummy gather at t~0: triggers the ~6us Q7 IRAM library
            # load so it overlaps the constant DMAs instead of delaying the
            # first real gather.
            warm = wpool.tile([P, D], bf16, tag="warm")
            nc.gpsimd.dma_gather(
                out_ap=warm[:].rearrange("p (t e) -> p t e", e=D),
                in_ap=ftab[0:cfg.wbounds[1], :],
                idxs_ap=idxt0[:, 0:1],
                num_idxs=16, num_idxs_reg=warm_reg, elem_size=D,
                single_packet=True, queue_num=0)

            for s, ks in enumerate(sb_ranges):
                # gather calls for this superbatch, one per source window.
                # queue rotates with s so the fat windows (5-tile budgets)
                # spread across the 4 Q7 desc-gen pairs over time; the
                # one-superbatch-deep pipeline absorbs the per-sb stagger.
                gts = {}
                for g in range(ng):
                    off, size = call_offsets[(s, g)]
                    if size == 0:
                        continue
                    gt = gpoolf.tile([P, (size // P) * D], bf16, tag=f"g{g}")
                    gt3 = gt[:].rearrange("p (t e) -> p t e", e=D)
                    glo = cfg.wbounds[g]
                    ghi = min(cfg.wbounds[g + 1], N)
                    if os.environ.get("GCN_SKIP_GATHER"):
                        nc.vector.memset(gt[:, 0:1], 0.0)
                    else:
                        q = (g + s) % 4
                        nc.gpsimd.dma_gather(
                            out_ap=gt3,
                            in_ap=ftab[glo:ghi, :],
                            idxs_ap=idx_slice(s, off, size),
                            num_idxs=size,
                            num_idxs_reg=size_regs[size],
                            elem_size=D,
                            # >64 descs/engine (1024 idx) overflows the SDMA
                            # packet limit when coalesced into one packet
                            single_packet=(size <= 1024),
                            queue_num=q,
                        )
                    gts[g] = (gt3, off // P)

                # one-hot tiles A[e, d] = (dstrel[e] == d), one broadcast
                # tensor_tensor per cell. Kept per-cell (not per-superbatch):
                # a single long 2-input DVE op holds both SBUF ports of the
                # POOL slot and locks the Q7 desc-gen out of SBUF — shorter
                # builds leave gaps for the gather's descriptor writes.
                abs_ = {}
                for g in range(ng):
                    off, size = call_offsets[(s, g)]
                    if size == 0:
                        continue
                    c0 = off // P
                    tkc = size // P
                    ab = apool.tile([P, tkc * P], bf16, tag=f"ab{g}")
                    d_b = dstt[:, c0:c0 + tkc].to_broadcast([P, tkc, P])
                    iap = iotap[:]
                    i_b = bass.AP(iap.tensor, iap.offset,
                                  [iap.ap[0], [0, tkc], iap.ap[1]])
                    nc.vector.tensor_tensor(
                        out=ab[:].rearrange("p (t d) -> p t d", d=P),
                        in0=i_b, in1=d_b, op=mybir.AluOpType.is_equal)
                    abs_[g] = (ab, c0)

                # per-sb degree slice (keeps the 24.5KB deg row out of SBUF)
                nks = len(ks)
                degsb = degp.tile([1, nks * P], bf16, tag="deg")
                nc.sync.dma_start(out=degsb[:],
                                  in_=degrow[0:1, ks.start * P:
                                             (ks.start + nks) * P])

                for k in ks:
                    ps = psag.tile([P, D], f32, tag="agg")
                    agg_ap = ps[:, 0:D]
                    ntiles = int(T[k].sum())
                    ti = 0
                    for g in range(ng):
                        tk = int(T[k][g])
                        if tk == 0:
                            continue
                        gt3, gcol0 = gts[g]
                        ab, acol0 = abs_[g]
                        c0 = tile_col[(k, g)]
                        for t in range(tk):
                            lc = c0 + t - gcol0  # column within gather tile
                            la = c0 + t - acol0  # column within A tile
                            amat = ab[:, la * P:(la + 1) * P]
                            first, last = ti == 0, ti == ntiles - 1
                            nc.tensor.matmul(agg_ap, lhsT=amat,
                                             rhs=gt3[:, lc, :],
                                             start=first, stop=last)
                            ti += 1

                    # FC for this block: out_blk = agg @ W + deg (x) b
                    aggs = wpool.tile([P, D], f32, tag="aggs")
                    nc.scalar.copy(out=aggs[:], in_=agg_ap)
                    po = psout.tile([P, DO], f32, tag="po")
                    for c in range(kchunks):
                        pt = pstr.tile([P, P], f32, tag="pt")
                        nc.tensor.transpose(pt[:], aggs[:, c * P:(c + 1) * P],
                                            identt[:])
                        aT = wpool.tile([P, P], f32, tag="aT")
                        nc.scalar.copy(out=aT[:], in_=pt[:])
                        nc.tensor.matmul(po[:], lhsT=aT[:], rhs=wts[c][:],
                                         start=(c == 0), stop=False)
                    # bias: deg (x) b as a rank-1 (contraction=1) matmul
                    j = k - ks.start
                    nc.tensor.matmul(po[:], lhsT=degsb[0:1, j * P:(j + 1) * P],
                                     rhs=bt[0:1, :], start=False, stop=True)
                    rows = min(P, cfg.npc - k * P)
                    outt = wpool.tile([P, DO], f32, tag="outt")
                    nc.scalar.copy(out=outt[:], in_=po[:])
                    nc.sync.dma_start(out=out[k * P:k * P + rows, :],
                                      in_=outt[:rows, :])
    return nc


def _run_spmd(nc, in_maps, trace=False):
    from concourse.bass_utils import run_bass_kernel_spmd
    return run_bass_kernel_spmd(nc, in_maps, list(range(len(in_maps))),
                                trace=trace)


_PROGRAM_CACHE = {}


def gcn_kernel(feature, W, b, src, dst, cfg=FULL_CFG, trace=False):
    in_maps, meta = _prep_host(feature, W, b, src, dst, cfg)
    key = (cfg.N, meta["tot"], tuple(np.asarray(meta["T"]).ravel().tolist()))
    nc = _PROGRAM_CACHE.get(key)
    if nc is None:
        nc = _build_program(cfg, meta)
        nc.finalize()
        _PROGRAM_CACHE[key] = nc
    res = _run_spmd(nc, in_maps, trace=trace)
    outs = [res.results[m]["out"] for m in range(cfg.ncores)]
    full = np.concatenate(outs, axis=0).astype(np.float32)
    return full, res


def kernel(**inputs):
    feature = np.asarray(inputs["feature"], dtype=np.float32)
    W = np.asarray(inputs["W"], dtype=np.float32)
    b = np.asarray(inputs["b"], dtype=np.float32)
    src = np.asarray(inputs["src"], dtype=np.int32)
    dst = np.asarray(inputs["dst"], dtype=np.int32)
    full, _ = gcn_kernel(feature, W, b, src, dst, FULL_CFG)
    return full


# revision 57
# speedup vs baseline: 1.3568x; 1.3568x over previous
"""GCN layer (fc + gather/scatter-sum) on 8 trn2 NeuronCores.

Math identity used: out = segment_sum(take(feature @ W + b, src), dst)
                        = segment_sum(take(feature, src), dst) @ W + deg * b
so the FC is folded AFTER edge aggregation. Each core owns a contiguous
range of dst nodes; its edges are host-sorted by (dst-block, src-window).
Feature rows are gathered as bf16 (512B rows) from a replicated DRAM table
with the custom SWDGE dma_gather (1 descriptor per edge). Desc-gen at
~8-10ns/desc per Q7 core pair (x4 pairs, one per SWDGE queue) is the
kernel's critical path, so: src windows are sized {27500,27500,22500,
22500} to land per-(block,window) edge counts just under {5,5,4,4} tile
multiples (fewer padded slots than equal windows); the fat windows rotate
across queues per superbatch; per-call num_idxs registers are allocated
per DISTINCT size and loaded once (a per-call reg_mov creates a WAR stall
that serializes the pairs); superbatch 0's idx slice uploads before the
bulk index table. Per 128-dst-node block, a one-hot A[e,d] =
(dst_rel[e] == d) is matmul'ed with the gathered rows into a PSUM
accumulator; A is built per (superbatch, window) cell in one broadcast
DVE is_equal — kept cell-sized because long 2-input DVE ops hold both
SBUF ports of the POOL slot and stall Q7 desc-gen. The block FC runs
on-chip: PE transpose, PSUM->SBUF copies on the otherwise-idle ACT
engine, two 128-contraction matmuls with W, and bias = deg (x) b folded
in as a rank-1 matmul (lhsT = deg row slice, rhs = b row) accumulated
into the same PSUM tile.
"""

import os
import numpy as np
import ml_dtypes

import concourse.bass as bass
import concourse.bacc as bacc
import concourse.mybir as mybir
from concourse import tile
from concourse import library_config

P = 128
NCORES = 8


def _patch_tile_exit():
    """The walrus build in this container rejects two constructs Tile emits
    at TileContext exit: a Drain carrying more than one sync wait ("Too many
    sync wait commands") and the sem_clear InstISA ("ISA wrong length").
    Replace the exit sequence with equivalent one-wait-per-Drain chains and
    skip the semaphore clears (fine for single-execution NEFFs)."""
    import bass_rust
    from concourse.vector_clock import ScopedClock

    def _drain_and_barrier(self, tick_clock, wait_clock):
        drain_inst = self.nc.sync.drain()
        wait_clock.add_sem_waits(
            drain_inst.ins, ScopedClock({None: tick_clock.global_clock})
        )
        si = drain_inst.ins.sync_info
        if si is not None and len(si.on_wait) > 1:
            waits = list(si.on_wait)
            drain_inst.ins.sync_info = bass_rust.SyncInfo(
                on_wait=waits[:1], on_update=list(si.on_update))
            for w in waits[1:]:
                extra = self.nc.sync.drain()
                extra.ins.sync_info = bass_rust.SyncInfo(
                    on_wait=[w], on_update=[])
        self.nc.all_engine_barrier()
        popped = self.nc._tile_sem_poison_stack.pop()
        assert popped is self._sem_poison
        self.nc.all_engine_barrier()

    tile.TileContext._drain_and_barrier = _drain_and_barrier


_patch_tile_exit()


class Cfg:
    def __init__(self, n_nodes, d_in, d_out, ncores, wbounds, sb_blocks):
        self.N = n_nodes
        self.D = d_in
        self.DO = d_out
        self.ncores = ncores
        # src window bounds; sizes tuned so per-(block,window) edge counts
        # land just under tile multiples (5,5,4,4 tiles) -> fewer padded
        # gather slots than equal windows (which ceil to 5 tiles each)
        self.wbounds = wbounds
        self.ngroups = len(wbounds) - 1
        assert max(b - a for a, b in zip(wbounds, wbounds[1:])) <= 32768
        self.npc = n_nodes // ncores       # nodes per core
        self.nblk = (self.npc + P - 1) // P
        self.sb = sb_blocks                # blocks per superbatch (psum + gather call)


FULL_CFG = Cfg(n_nodes=100000, d_in=256, d_out=64, ncores=8,
               wbounds=(0, 27500, 55000, 77500, 100000), sb_blocks=4)


def _prep_host(feature, W, b, src, dst, cfg):
    """Shard + sort edges, build per-core slot arrays and shared tile budgets."""
    N, npc, nblk, ng = cfg.N, cfg.npc, cfg.nblk, cfg.ngroups
    src = np.asarray(src, dtype=np.int64)
    dst = np.asarray(dst, dtype=np.int64)

    per_core = []
    counts = np.zeros((cfg.ncores, nblk, ng), dtype=np.int64)
    for m in range(cfg.ncores):
        lo, hi = m * npc, (m + 1) * npc
        mask = (dst >= lo) & (dst < hi)
        es = src[mask]
        ed = dst[mask] - lo
        blk = ed >> 7
        grp = np.searchsorted(np.asarray(cfg.wbounds), es, side="right") - 1
        order = np.lexsort((es, grp, blk))
        es, ed, blk, grp = es[order], ed[order], blk[order], grp[order]
        np.add.at(counts[m], (blk, grp), 1)
        per_core.append((es, ed, blk, grp))

    # static tile budgets per (block, group): max over cores, >=1 tile per block
    T = np.ceil(counts / P).astype(np.int64).max(axis=0)  # [nblk, ng]
    for k in range(nblk):
        if T[k].sum() == 0:
            T[k][0] = 1

    # slot layout in call order: superbatch -> group -> block -> tiles.
    # The last few superbatches shrink to 2 blocks so the end-of-kernel
    # pipeline drain (compute lags desc-gen by ~2 superbatches) is short.
    sb_ranges = []
    k0 = 0
    while k0 < nblk:
        rem = nblk - k0
        step = cfg.sb if rem > 6 else 2
        sb_ranges.append(range(k0, min(k0 + step, nblk)))
        k0 += step
    nsb = len(sb_ranges)
    call_offsets = {}   # (sb, g) -> (slot_start, slot_count)
    tile_col = {}       # (k, g) -> first slot column (slot_start // P)
    pos = 0
    for s, ks in enumerate(sb_ranges):
        for g in range(ng):
            size = int(sum(T[k][g] for k in ks)) * P
            call_offsets[(s, g)] = (pos, size)
            for k in ks:
                tile_col[(k, g)] = pos // P
                pos += int(T[k][g]) * P
    tot = pos
    assert tot % P == 0

    in_maps = []
    ftab = np.ascontiguousarray(feature.astype(ml_dtypes.bfloat16))
    wmat = np.ascontiguousarray(W.astype(np.float32))
    brow = np.ascontiguousarray(b.astype(ml_dtypes.bfloat16)[None, :])
    iota = np.ascontiguousarray(
        np.tile(np.arange(P, dtype=np.float32)[None, :], (P, 1)).astype(
            ml_dtypes.bfloat16))
    ident = np.eye(P, dtype=np.float32)

    for m in range(cfg.ncores):
        es, ed, blk, grp = per_core[m]
        idx_arr = np.zeros(tot, dtype=np.int16)
        dst_arr = np.full(tot, -1.0, dtype=np.float32)
        # edges of (k, g) start at slot tile_col[(k,g)]*P
        bounds = np.searchsorted(blk * ng + grp, np.arange(nblk * ng + 1))
        for k in range(nblk):
            for g in range(ng):
                a, bnd = bounds[k * ng + g], bounds[k * ng + g + 1]
                n = bnd - a
                if n == 0:
                    continue
                s0 = tile_col[(k, g)] * P
                idx_arr[s0:s0 + n] = (es[a:bnd] - cfg.wbounds[g]).astype(np.int16)
                dst_arr[s0:s0 + n] = (ed[a:bnd] - k * P).astype(np.float32)
        # ucode rx/tx cores read the idx wrap from different 16-partition
        # groups (queue-dependent): replicate it across all 128 partitions
        idx16 = np.ascontiguousarray(
            np.tile(idx_arr.reshape(tot // 16, 16).T, (P // 16, 1)))
        dstrel = np.ascontiguousarray(
            dst_arr.reshape(tot // P, P).T.astype(ml_dtypes.bfloat16))
        deg = np.zeros(nblk * P, dtype=np.float32)
        np.add.at(deg, ed, 1.0)
        # single partition-0 row; [1,128] lhsT slices per block (bf16 exact
        # for small integer degrees)
        degrow = np.ascontiguousarray(deg[None, :].astype(ml_dtypes.bfloat16))
        in_maps.append({
            "ftab": ftab, "idx16": idx16, "dstrel": dstrel,
            "wmat": wmat, "brow": brow, "iota": iota, "ident": ident,
            "degrow": degrow,
        })

    meta = dict(T=T, call_offsets=call_offsets, tile_col=tile_col, tot=tot,
                nsb=nsb, sb_ranges=sb_ranges)
    return in_maps, meta


def _build_program(cfg, meta):
    N, D, DO, nblk, ng = cfg.N, cfg.D, cfg.DO, cfg.nblk, cfg.ngroups
    T, call_offsets, tile_col = meta["T"], meta["call_offsets"], meta["tile_col"]
    tot, nsb, sb_ranges = meta["tot"], meta["nsb"], meta["sb_ranges"]
    bf16, f32, i16 = mybir.dt.bfloat16, mybir.dt.float32, mybir.dt.int16

    nc = bacc.Bacc(None, target_bir_lowering=False, num_swdge_queues=4)
    ftab = nc.dram_tensor("ftab", [N, D], bf16, kind="ExternalInput")
    idx16 = nc.dram_tensor("idx16", [P, tot // 16], i16, kind="ExternalInput")
    dstrel = nc.dram_tensor("dstrel", [P, tot // P], bf16, kind="ExternalInput")
    wmat = nc.dram_tensor("wmat", [D, DO], f32, kind="ExternalInput")
    brow = nc.dram_tensor("brow", [1, DO], bf16, kind="ExternalInput")
    iota = nc.dram_tensor("iota", [P, P], bf16, kind="ExternalInput")
    ident = nc.dram_tensor("ident", [P, P], f32, kind="ExternalInput")
    degrow = nc.dram_tensor("degrow", [1, nblk * P], bf16, kind="ExternalInput")
    out = nc.dram_tensor("out", [cfg.npc, DO], f32, kind="ExternalOutput")

    kchunks = D // P  # 128-contraction chunks of the FC

    with tile.TileContext(nc) as tc:
        with (
            tc.tile_pool(name="const", bufs=1) as cpool,
            tc.tile_pool(name="gathf", bufs=3) as gpoolf,
            tc.tile_pool(name="degp", bufs=2) as degp,
            tc.tile_pool(name="amat", bufs=2) as apool,
            tc.tile_pool(name="work", bufs=4) as wpool,
            tc.tile_pool(name="psag", bufs=cfg.sb, space="PSUM") as psag,
            tc.tile_pool(name="pstr", bufs=2, space="PSUM") as pstr,
            tc.tile_pool(name="psout", bufs=2, space="PSUM") as psout,
        ):
            # one Pool register per DISTINCT call size, loaded once up front.
            # A per-call reg_mov would create a WAR hazard on the register
            # (next mov waits for the previous gather reading it), and the
            # in-order Pool FIFO then stalls dispatch of the other queues'
            # gathers — killing the 4-way Q7-pair desc-gen concurrency.
            distinct_sizes = sorted({sz for (_o, sz) in call_offsets.values()
                                     if sz > 0})
            size_regs = {}
            for sz in distinct_sizes:
                r = nc.alloc_register(mybir.EngineType.Pool, f"gsz{sz}")
                nc.gpsimd.reg_mov(r, sz)
                size_regs[sz] = r
            warm_reg = nc.alloc_register(mybir.EngineType.Pool, "gszwarm")
            nc.gpsimd.reg_mov(warm_reg, 16)
            iotat = cpool.tile([P, P], bf16)
            nc.sync.dma_start(out=iotat[:], in_=iota[:])
            identt = cpool.tile([P, P], f32)
            nc.sync.dma_start(out=identt[:], in_=ident[:])

            bt = cpool.tile([1, DO], bf16)
            nc.sync.dma_start(out=bt[:], in_=brow[:])
            wts = []
            for c in range(kchunks):
                wt = cpool.tile([P, DO], f32, tag=f"w{c}")
                nc.sync.dma_start(out=wt[:], in_=wmat[c * P:(c + 1) * P, :])
                wts.append(wt)
            # idx table split: superbatch 0's slice is a separate small tile
            # loaded first so sb0's gathers don't stall behind the full
            # 3.6MB index upload; dstt likewise precedes the bulk idx DMA.
            sb0_end = sum(call_offsets[(0, g)][1] for g in range(ng))
            idxt0 = cpool.tile([P, sb0_end // 16], i16)
            nc.sync.dma_start(out=idxt0[:], in_=idx16[:, 0:sb0_end // 16])
            dstt = cpool.tile([P, tot // P], bf16)
            nc.sync.dma_start(out=dstt[:], in_=dstrel[:])
            idxt = cpool.tile([P, tot // 16], i16)
            nc.sync.dma_start(out=idxt[:, sb0_end // 16:],
                              in_=idx16[:, sb0_end // 16:])

            def idx_slice(s, off, size):
                if s == 0:
                    return idxt0[:, off // 16:(off + size) // 16]
                return idxt[:, off // 16:(off + size) // 16]

            # tiny dummy gather at t~0: triggers the ~6us Q7 IRAM library
            # load so it overlaps the constant DMAs instead of delaying the
            # first real gather.
            warm = wpool.tile([P, D], bf16, tag="warm")
            nc.gpsimd.dma_gather(
                out_ap=warm[:].rearrange("p (t e) -> p t e", e=D),
                in_ap=ftab[0:cfg.wbounds[1], :],
                idxs_ap=idxt0[:, 0:1],
                num_idxs=16, num_idxs_reg=warm_reg, elem_size=D,
                single_packet=True, queue_num=0)

            for s, ks in enumerate(sb_ranges):
                # gather calls for this superbatch, one per source window.
                # queue rotates with s so the fat windows (5-tile budgets)
                # spread across the 4 Q7 desc-gen pairs over time; the
                # one-superbatch-deep pipeline absorbs the per-sb stagger.
                gts = {}
                for g in range(ng):
                    off, size = call_offsets[(s, g)]
                    if size == 0:
                        continue
                    gt = gpoolf.tile([P, (size // P) * D], bf16, tag=f"g{g}")
                    gt3 = gt[:].rearrange("p (t e) -> p t e", e=D)
                    glo = cfg.wbounds[g]
                    ghi = min(cfg.wbounds[g + 1], N)
                    if os.environ.get("GCN_SKIP_GATHER"):
                        nc.vector.memset(gt[:, 0:1], 0.0)
                    else:
                        q = (g + s) % 4
                        nc.gpsimd.dma_gather(
                            out_ap=gt3,
                            in_ap=ftab[glo:ghi, :],
                            idxs_ap=idx_slice(s, off, size),
                            num_idxs=size,
                            num_idxs_reg=size_regs[size],
                            elem_size=D,
                            # >64 descs/engine (1024 idx) overflows the SDMA
                            # packet limit when coalesced into one packet
                            single_packet=(size <= 1024),
                            queue_num=q,
                        )
                    gts[g] = (gt3, off // P)

                # one-hot tiles A[e, d] = (dstrel[e] == d), one broadcast
                # tensor_tensor per cell. Kept per-cell (not per-superbatch):
                # a single long 2-input DVE op holds both SBUF ports of the
                # POOL slot and locks the Q7 desc-gen out of SBUF — shorter
                # builds leave gaps for the gather's descriptor writes.
                abs_ = {}
                for g in range(ng):
                    off, size = call_offsets[(s, g)]
                    if size == 0:
                        continue
                    c0 = off // P
                    tkc = size // P
                    ab = apool.tile([P, tkc * P], bf16, tag=f"ab{g}")
                    d_b = dstt[:, c0:c0 + tkc].to_broadcast([P, tkc, P])
                    iap = iotat[:]
                    i_b = bass.AP(iap.tensor, iap.offset,
                                  [iap.ap[0], [0, tkc], iap.ap[1]])
                    nc.vector.tensor_tensor(
                        out=ab[:].rearrange("p (t d) -> p t d", d=P),
                        in0=i_b, in1=d_b, op=mybir.AluOpType.is_equal)
                    abs_[g] = (ab, c0)

                # per-sb degree slice (keeps the 24.5KB deg row out of SBUF)
                nks = len(ks)
                degsb = degp.tile([1, nks * P], bf16, tag="deg")
                nc.sync.dma_start(out=degsb[:],
                                  in_=degrow[0:1, ks.start * P:
                                             (ks.start + nks) * P])

                for k in ks:
                    ps = psag.tile([P, D], f32, tag="agg")
                    agg_ap = ps[:, 0:D]
                    ntiles = int(T[k].sum())
                    ti = 0
                    for g in range(ng):
                        tk = int(T[k][g])
                        if tk == 0:
                            continue
                        gt3, gcol0 = gts[g]
                        ab, acol0 = abs_[g]
                        c0 = tile_col[(k, g)]
                        for t in range(tk):
                            lc = c0 + t - gcol0  # column within gather tile
                            la = c0 + t - acol0  # column within A tile
                            amat = ab[:, la * P:(la + 1) * P]
                            first, last = ti == 0, ti == ntiles - 1
                            nc.tensor.matmul(agg_ap, lhsT=amat,
                                             rhs=gt3[:, lc, :],
                                             start=first, stop=last)
                            ti += 1

                    # FC for this block: out_blk = agg @ W + deg (x) b
                    aggs = wpool.tile([P, D], f32, tag="aggs")
                    nc.scalar.copy(out=aggs[:], in_=agg_ap)
                    po = psout.tile([P, DO], f32, tag="po")
                    for c in range(kchunks):
                        pt = pstr.tile([P, P], f32, tag="pt")
                        nc.tensor.transpose(pt[:], aggs[:, c * P:(c + 1) * P],
                                            identt[:])
                        aT = wpool.tile([P, P], f32, tag="aT")
                        nc.scalar.copy(out=aT[:], in_=pt[:])
                        nc.tensor.matmul(po[:], lhsT=aT[:], rhs=wts[c][:],
                                         start=(c == 0), stop=False)
                    # bias: deg (x) b as a rank-1 (contraction=1) matmul
                    j = k - ks.start
                    nc.tensor.matmul(po[:], lhsT=degsb[0:1, j * P:(j + 1) * P],
                                     rhs=bt[0:1, :], start=False, stop=True)
                    rows = min(P, cfg.npc - k * P)
                    outt = wpool.tile([P, DO], f32, tag="outt")
                    nc.scalar.copy(out=outt[:], in_=po[:])
                    nc.sync.dma_start(out=out[k * P:k * P + rows, :],
                                      in_=outt[:rows, :])
    return nc


def _run_spmd(nc, in_maps, trace=False):
    from concourse.bass_utils import run_bass_kernel_spmd
    return run_bass_kernel_spmd(nc, in_maps, list(range(len(in_maps))),
                                trace=trace)


_PROGRAM_CACHE = {}


def gcn_kernel(feature, W, b, src, dst, cfg=FULL_CFG, trace=False):
    in_maps, meta = _prep_host(feature, W, b, src, dst, cfg)
    key = (cfg.N, meta["tot"], tuple(np.asarray(meta["T"]).ravel().tolist()))
    nc = _PROGRAM_CACHE.get(key)
    if nc is None:
        nc = _build_program(cfg, meta)
        nc.finalize()
        _PROGRAM_CACHE[key] = nc
    res = _run_spmd(nc, in_maps, trace=trace)
    outs = [res.results[m]["out"] for m in range(cfg.ncores)]
    full = np.concatenate(outs, axis=0).astype(np.float32)
    return full, res


def kernel(**inputs):
    feature = np.asarray(inputs["feature"], dtype=np.float32)
    W = np.asarray(inputs["W"], dtype=np.float32)
    b = np.asarray(inputs["b"], dtype=np.float32)
    src = np.asarray(inputs["src"], dtype=np.int32)
    dst = np.asarray(inputs["dst"], dtype=np.int32)
    full, _ = gcn_kernel(feature, W, b, src, dst, FULL_CFG)
    return full


# revision 59
# speedup vs baseline: 1.3626x; 1.0042x over previous
"""GCN layer (fc + gather/scatter-sum) on 8 trn2 NeuronCores.

Math identity used: out = segment_sum(take(feature @ W + b, src), dst)
                        = segment_sum(take(feature, src), dst) @ W + deg * b
so the FC is folded AFTER edge aggregation. Each core owns a contiguous
range of dst nodes; its edges are host-sorted by (dst-block, src-window).
Feature rows are gathered as bf16 (512B rows) from a replicated DRAM table
with the custom SWDGE dma_gather (1 descriptor per edge). Desc-gen at
~8-10ns/desc per Q7 core pair (x4 pairs, one per SWDGE queue) is the
kernel's critical path, so: src windows are sized {27500,27500,22500,
22500} to land per-(block,window) edge counts just under {5,5,4,4} tile
multiples (fewer padded slots than equal windows); the fat windows rotate
across queues per superbatch; per-call num_idxs registers are allocated
per DISTINCT size and loaded once (a per-call reg_mov creates a WAR stall
that serializes the pairs); superbatch 0's idx slice uploads before the
bulk index table. Per 128-dst-node block, a one-hot A[e,d] =
(dst_rel[e] == d) is matmul'ed with the gathered rows into a PSUM
accumulator; A is built per (superbatch, window) cell in one broadcast
DVE is_equal — kept cell-sized because long 2-input DVE ops hold both
SBUF ports of the POOL slot and stall Q7 desc-gen. The block FC runs
on-chip: PE transpose, PSUM->SBUF copies on the otherwise-idle ACT
engine, two 128-contraction matmuls with W, and bias = deg (x) b folded
in as a rank-1 matmul (lhsT = deg row slice, rhs = b row) accumulated
into the same PSUM tile.
"""

import os
import numpy as np
import ml_dtypes

import concourse.bass as bass
import concourse.bacc as bacc
import concourse.mybir as mybir
from concourse import tile
from concourse import library_config

P = 128
NCORES = 8


def _patch_tile_exit():
    """The walrus build in this container rejects two constructs Tile emits
    at TileContext exit: a Drain carrying more than one sync wait ("Too many
    sync wait commands") and the sem_clear InstISA ("ISA wrong length").
    Replace the exit sequence with equivalent one-wait-per-Drain chains and
    skip the semaphore clears (fine for single-execution NEFFs)."""
    import bass_rust
    from concourse.vector_clock import ScopedClock

    def _drain_and_barrier(self, tick_clock, wait_clock):
        drain_inst = self.nc.sync.drain()
        wait_clock.add_sem_waits(
            drain_inst.ins, ScopedClock({None: tick_clock.global_clock})
        )
        si = drain_inst.ins.sync_info
        if si is not None and len(si.on_wait) > 1:
            waits = list(si.on_wait)
            drain_inst.ins.sync_info = bass_rust.SyncInfo(
                on_wait=waits[:1], on_update=list(si.on_update))
            for w in waits[1:]:
                extra = self.nc.sync.drain()
                extra.ins.sync_info = bass_rust.SyncInfo(
                    on_wait=[w], on_update=[])
        self.nc.all_engine_barrier()
        popped = self.nc._tile_sem_poison_stack.pop()
        assert popped is self._sem_poison
        self.nc.all_engine_barrier()

    tile.TileContext._drain_and_barrier = _drain_and_barrier


_patch_tile_exit()


class Cfg:
    def __init__(self, n_nodes, d_in, d_out, ncores, wbounds, sb_blocks):
        self.N = n_nodes
        self.D = d_in
        self.DO = d_out
        self.ncores = ncores
        # src window bounds; sizes tuned so per-(block,window) edge counts
        # land just under tile multiples (5,5,4,4 tiles) -> fewer padded
        # gather slots than equal windows (which ceil to 5 tiles each)
        self.wbounds = wbounds
        self.ngroups = len(wbounds) - 1
        assert max(b - a for a, b in zip(wbounds, wbounds[1:])) <= 32768
        self.npc = n_nodes // ncores       # nodes per core
        self.nblk = (self.npc + P - 1) // P
        self.sb = sb_blocks                # blocks per superbatch (psum + gather call)


FULL_CFG = Cfg(n_nodes=100000, d_in=256, d_out=64, ncores=8,
               wbounds=(0, 27500, 55000, 77500, 100000), sb_blocks=4)


def _prep_host(feature, W, b, src, dst, cfg):
    """Shard + sort edges, build per-core slot arrays and shared tile budgets."""
    N, npc, nblk, ng = cfg.N, cfg.npc, cfg.nblk, cfg.ngroups
    src = np.asarray(src, dtype=np.int64)
    dst = np.asarray(dst, dtype=np.int64)

    per_core = []
    counts = np.zeros((cfg.ncores, nblk, ng), dtype=np.int64)
    for m in range(cfg.ncores):
        lo, hi = m * npc, (m + 1) * npc
        mask = (dst >= lo) & (dst < hi)
        es = src[mask]
        ed = dst[mask] - lo
        blk = ed >> 7
        grp = np.searchsorted(np.asarray(cfg.wbounds), es, side="right") - 1
        order = np.lexsort((es, grp, blk))
        es, ed, blk, grp = es[order], ed[order], blk[order], grp[order]
        np.add.at(counts[m], (blk, grp), 1)
        per_core.append((es, ed, blk, grp))

    # static tile budgets per (block, group): max over cores, >=1 tile per block
    T = np.ceil(counts / P).astype(np.int64).max(axis=0)  # [nblk, ng]
    for k in range(nblk):
        if T[k].sum() == 0:
            T[k][0] = 1

    # slot layout in call order: superbatch -> group -> block -> tiles.
    # The last few superbatches shrink to 2 blocks so the end-of-kernel
    # pipeline drain (compute lags desc-gen by ~2 superbatches) is short.
    sb_ranges = []
    k0 = 0
    while k0 < nblk:
        rem = nblk - k0
        step = cfg.sb if rem > 6 else 2
        sb_ranges.append(range(k0, min(k0 + step, nblk)))
        k0 += step
    nsb = len(sb_ranges)
    call_offsets = {}   # (sb, g) -> (slot_start, slot_count)
    tile_col = {}       # (k, g) -> first slot column (slot_start // P)
    pos = 0
    for s, ks in enumerate(sb_ranges):
        for g in range(ng):
            size = int(sum(T[k][g] for k in ks)) * P
            call_offsets[(s, g)] = (pos, size)
            for k in ks:
                tile_col[(k, g)] = pos // P
                pos += int(T[k][g]) * P
    tot = pos
    assert tot % P == 0

    in_maps = []
    ftab = np.ascontiguousarray(feature.astype(ml_dtypes.bfloat16))
    wmat = np.ascontiguousarray(W.astype(np.float32))
    brow = np.ascontiguousarray(b.astype(ml_dtypes.bfloat16)[None, :])
    iota = np.ascontiguousarray(
        np.tile(np.arange(P, dtype=np.float32)[None, :], (P, 1)).astype(
            ml_dtypes.bfloat16))
    ident = np.eye(P, dtype=np.float32)

    for m in range(cfg.ncores):
        es, ed, blk, grp = per_core[m]
        idx_arr = np.zeros(tot, dtype=np.int16)
        dst_arr = np.full(tot, -1.0, dtype=np.float32)
        # edges of (k, g) start at slot tile_col[(k,g)]*P
        bounds = np.searchsorted(blk * ng + grp, np.arange(nblk * ng + 1))
        for k in range(nblk):
            for g in range(ng):
                a, bnd = bounds[k * ng + g], bounds[k * ng + g + 1]
                n = bnd - a
                if n == 0:
                    continue
                s0 = tile_col[(k, g)] * P
                idx_arr[s0:s0 + n] = (es[a:bnd] - cfg.wbounds[g]).astype(np.int16)
                dst_arr[s0:s0 + n] = (ed[a:bnd] - k * P).astype(np.float32)
        # ucode rx/tx cores read the idx wrap from different 16-partition
        # groups (queue-dependent): replicate it across all 128 partitions
        idx16 = np.ascontiguousarray(
            np.tile(idx_arr.reshape(tot // 16, 16).T, (P // 16, 1)))
        dstrel = np.ascontiguousarray(
            dst_arr.reshape(tot // P, P).T.astype(ml_dtypes.bfloat16))
        deg = np.zeros(nblk * P, dtype=np.float32)
        np.add.at(deg, ed, 1.0)
        # single partition-0 row; [1,128] lhsT slices per block (bf16 exact
        # for small integer degrees)
        degrow = np.ascontiguousarray(deg[None, :].astype(ml_dtypes.bfloat16))
        in_maps.append({
            "ftab": ftab, "idx16": idx16, "dstrel": dstrel,
            "wmat": wmat, "brow": brow, "iota": iota, "ident": ident,
            "degrow": degrow,
        })

    meta = dict(T=T, call_offsets=call_offsets, tile_col=tile_col, tot=tot,
                nsb=nsb, sb_ranges=sb_ranges)
    return in_maps, meta


def _build_program(cfg, meta):
    N, D, DO, nblk, ng = cfg.N, cfg.D, cfg.DO, cfg.nblk, cfg.ngroups
    T, call_offsets, tile_col = meta["T"], meta["call_offsets"], meta["tile_col"]
    tot, nsb, sb_ranges = meta["tot"], meta["nsb"], meta["sb_ranges"]
    bf16, f32, i16 = mybir.dt.bfloat16, mybir.dt.float32, mybir.dt.int16

    nc = bacc.Bacc(None, target_bir_lowering=False, num_swdge_queues=4)
    ftab = nc.dram_tensor("ftab", [N, D], bf16, kind="ExternalInput")
    idx16 = nc.dram_tensor("idx16", [P, tot // 16], i16, kind="ExternalInput")
    dstrel = nc.dram_tensor("dstrel", [P, tot // P], bf16, kind="ExternalInput")
    wmat = nc.dram_tensor("wmat", [D, DO], f32, kind="ExternalInput")
    brow = nc.dram_tensor("brow", [1, DO], bf16, kind="ExternalInput")
    iota = nc.dram_tensor("iota", [P, P], bf16, kind="ExternalInput")
    ident = nc.dram_tensor("ident", [P, P], f32, kind="ExternalInput")
    degrow = nc.dram_tensor("degrow", [1, nblk * P], bf16, kind="ExternalInput")
    out = nc.dram_tensor("out", [cfg.npc, DO], f32, kind="ExternalOutput")

    kchunks = D // P  # 128-contraction chunks of the FC

    with tile.TileContext(nc) as tc:
        with (
            tc.tile_pool(name="const", bufs=1) as cpool,
            tc.tile_pool(name="gathf", bufs=3) as gpoolf,
            tc.tile_pool(name="degp", bufs=2) as degp,
            tc.tile_pool(name="amat", bufs=2) as apool,
            tc.tile_pool(name="work", bufs=4) as wpool,
            tc.tile_pool(name="psag", bufs=cfg.sb, space="PSUM") as psag,
            tc.tile_pool(name="pstr", bufs=2, space="PSUM") as pstr,
            tc.tile_pool(name="psout", bufs=2, space="PSUM") as psout,
        ):
            # one Pool register per DISTINCT call size, loaded once up front.
            # A per-call reg_mov would create a WAR hazard on the register
            # (next mov waits for the previous gather reading it), and the
            # in-order Pool FIFO then stalls dispatch of the other queues'
            # gathers — killing the 4-way Q7-pair desc-gen concurrency.
            distinct_sizes = sorted({sz for (_o, sz) in call_offsets.values()
                                     if sz > 0})
            size_regs = {}
            for sz in distinct_sizes:
                r = nc.alloc_register(mybir.EngineType.Pool, f"gsz{sz}")
                nc.gpsimd.reg_mov(r, sz)
                size_regs[sz] = r
            warm_reg = nc.alloc_register(mybir.EngineType.Pool, "gszwarm")
            nc.gpsimd.reg_mov(warm_reg, 16)
            iotat = cpool.tile([P, P], bf16)
            nc.sync.dma_start(out=iotat[:], in_=iota[:])
            identt = cpool.tile([P, P], f32)
            nc.sync.dma_start(out=identt[:], in_=ident[:])

            bt = cpool.tile([1, DO], bf16)
            nc.sync.dma_start(out=bt[:], in_=brow[:])
            wts = []
            for c in range(kchunks):
                wt = cpool.tile([P, DO], f32, tag=f"w{c}")
                nc.sync.dma_start(out=wt[:], in_=wmat[c * P:(c + 1) * P, :])
                wts.append(wt)
            # idx table split: superbatch 0's slice is a separate small tile
            # loaded first so sb0's gathers don't stall behind the full
            # 3.6MB index upload; dstt likewise precedes the bulk idx DMA.
            sb0_end = sum(call_offsets[(0, g)][1] for g in range(ng))
            idxt0 = cpool.tile([P, sb0_end // 16], i16)
            nc.sync.dma_start(out=idxt0[:], in_=idx16[:, 0:sb0_end // 16])
            dstt = cpool.tile([P, tot // P], bf16)
            nc.sync.dma_start(out=dstt[:], in_=dstrel[:])
            idxt = cpool.tile([P, tot // 16], i16)
            nc.sync.dma_start(out=idxt[:, sb0_end // 16:],
                              in_=idx16[:, sb0_end // 16:])

            def idx_slice(s, off, size):
                if s == 0:
                    return idxt0[:, off // 16:(off + size) // 16]
                return idxt[:, off // 16:(off + size) // 16]

            # tiny dummy gather at t~0: triggers the ~6us Q7 IRAM library
            # load so it overlaps the constant DMAs instead of delaying the
            # first real gather.
            warm = wpool.tile([P, D], bf16, tag="warm")
            nc.gpsimd.dma_gather(
                out_ap=warm[:].rearrange("p (t e) -> p t e", e=D),
                in_ap=ftab[0:cfg.wbounds[1], :],
                idxs_ap=idxt0[:, 0:1],
                num_idxs=16, num_idxs_reg=warm_reg, elem_size=D,
                single_packet=True, queue_num=0)

            for s, ks in enumerate(sb_ranges):
                # gather calls for this superbatch, one per source window.
                # queue rotates with s so the fat windows (5-tile budgets)
                # spread across the 4 Q7 desc-gen pairs over time; the
                # one-superbatch-deep pipeline absorbs the per-sb stagger.
                gts = {}
                for g in range(ng):
                    off, size = call_offsets[(s, g)]
                    if size == 0:
                        continue
                    gt = gpoolf.tile([P, (size // P) * D], bf16, tag=f"g{g}")
                    gt3 = gt[:].rearrange("p (t e) -> p t e", e=D)
                    glo = cfg.wbounds[g]
                    ghi = min(cfg.wbounds[g + 1], N)
                    if os.environ.get("GCN_SKIP_GATHER"):
                        nc.vector.memset(gt[:, 0:1], 0.0)
                    else:
                        q = (g + s) % 4
                        nc.gpsimd.dma_gather(
                            out_ap=gt3,
                            in_ap=ftab[glo:ghi, :],
                            idxs_ap=idx_slice(s, off, size),
                            num_idxs=size,
                            num_idxs_reg=size_regs[size],
                            elem_size=D,
                            # >64 descs/engine (1024 idx) overflows the SDMA
                            # packet limit when coalesced into one packet
                            single_packet=(size <= 1024),
                            queue_num=q,
                        )
                    gts[g] = (gt3, off // P)

                # one-hot tiles A[e, d] = (dstrel[e] == d), one broadcast
                # tensor_tensor per cell. Kept per-cell (not per-superbatch):
                # a single long 2-input DVE op holds both SBUF ports of the
                # POOL slot and locks the Q7 desc-gen out of SBUF — shorter
                # builds leave gaps for the gather's descriptor writes.
                abs_ = {}
                for g in range(ng):
                    off, size = call_offsets[(s, g)]
                    if size == 0:
                        continue
                    c0 = off // P
                    tkc = size // P
                    ab = apool.tile([P, tkc * P], bf16, tag=f"ab{g}")
                    d_b = dstt[:, c0:c0 + tkc].to_broadcast([P, tkc, P])
                    iap = iotat[:]
                    i_b = bass.AP(iap.tensor, iap.offset,
                                  [iap.ap[0], [0, tkc], iap.ap[1]])
                    nc.vector.tensor_tensor(
                        out=ab[:].rearrange("p (t d) -> p t d", d=P),
                        in0=i_b, in1=d_b, op=mybir.AluOpType.is_equal)
                    abs_[g] = (ab, c0)

                # per-sb degree slice (keeps the 24.5KB deg row out of SBUF)
                nks = len(ks)
                degsb = degp.tile([1, nks * P], bf16, tag="deg")
                nc.sync.dma_start(out=degsb[:],
                                  in_=degrow[0:1, ks.start * P:
                                             (ks.start + nks) * P])

                for k in ks:
                    ps = psag.tile([P, D], f32, tag="agg")
                    agg_ap = ps[:, 0:D]
                    ntiles = int(T[k].sum())
                    ti = 0
                    for g in range(ng):
                        tk = int(T[k][g])
                        if tk == 0:
                            continue
                        gt3, gcol0 = gts[g]
                        ab, acol0 = abs_[g]
                        c0 = tile_col[(k, g)]
                        for t in range(tk):
                            lc = c0 + t - gcol0  # column within gather tile
                            la = c0 + t - acol0  # column within A tile
                            amat = ab[:, la * P:(la + 1) * P]
                            first, last = ti == 0, ti == ntiles - 1
                            nc.tensor.matmul(agg_ap, lhsT=amat,
                                             rhs=gt3[:, lc, :],
                                             start=first, stop=last)
                            ti += 1

                    # FC for this block: out_blk = agg @ W + deg (x) b
                    aggs = wpool.tile([P, D], f32, tag="aggs")
                    nc.scalar.copy(out=aggs[:], in_=agg_ap)
                    po = psout.tile([P, DO], f32, tag="po")
                    for c in range(kchunks):
                        pt = pstr.tile([P, P], f32, tag="pt")
                        nc.tensor.transpose(pt[:], aggs[:, c * P:(c + 1) * P],
                                            identt[:])
                        aT = wpool.tile([P, P], f32, tag="aT")
                        nc.scalar.copy(out=aT[:], in_=pt[:])
                        nc.tensor.matmul(po[:], lhsT=aT[:], rhs=wts[c][:],
                                         start=(c == 0), stop=False)
                    # bias: deg (x) b as a rank-1 (contraction=1) matmul
                    j = k - ks.start
                    nc.tensor.matmul(po[:], lhsT=degsb[0:1, j * P:(j + 1) * P],
                                     rhs=bt[0:1, :], start=False, stop=True)
                    rows = min(P, cfg.npc - k * P)
                    outt = wpool.tile([P, DO], f32, tag="outt")
                    nc.scalar.copy(out=outt[:], in_=po[:])
                    nc.sync.dma_start(out=out[k * P:k * P + rows, :],
                                      in_=outt[:rows, :])
    return nc


def _run_spmd(nc, in_maps, trace=False):
    from concourse.bass_utils import run_bass_kernel_spmd
    return run_bass_kernel_spmd(nc, in_maps, list(range(len(in_maps))),
                                trace=trace)


_PROGRAM_CACHE = {}


def gcn_kernel(feature, W, b, src, dst, cfg=FULL_CFG, trace=False):
    in_maps, meta = _prep_host(feature, W, b, src, dst, cfg)
    key = (cfg.N, meta["tot"], tuple(np.asarray(meta["T"]).ravel().tolist()))
    nc = _PROGRAM_CACHE.get(key)
    if nc is None:
        nc = _build_program(cfg, meta)
        nc.finalize()
        _PROGRAM_CACHE[key] = nc
    res = _run_spmd(nc, in_maps, trace=trace)
    outs = [res.results[m]["out"] for m in range(cfg.ncores)]
    full = np.concatenate(outs, axis=0).astype(np.float32)
    return full, res


def kernel(**inputs):
    feature = np.asarray(inputs["feature"], dtype=np.float32)
    W = np.asarray(inputs["W"], dtype=np.float32)
    b = np.asarray(inputs["b"], dtype=np.float32)
    src = np.asarray(inputs["src"], dtype=np.int32)
    dst = np.asarray(inputs["dst"], dtype=np.int32)
    full, _ = gcn_kernel(feature, W, b, src, dst, FULL_CFG)
    return full
